# revision 7
# baseline (speedup 1.0000x reference)
"""Class-attention with GFSA reaction term — Trainium2 Bass kernel, 8 NeuronCores.

Math (reference):
    q,k,v = x@W{q,k,v}.T split into H=12 heads of 64
    A  = softmax(q k^T / 8)                  per (b,h), [N,N], N=577
    attn = A + lamb_h * (3*A@A - 2*A)
    out  = (attn @ v) @ Wp.T + bp

Key algebraic restructure (avoids the N^3 A@A entirely):
    out_head = (1-2l)*A@V + 3l*A@(A@V)
    With E = exp(logits) unnormalized and D = diag(rowsum(E)):
        U = E@V,  s = rowsum(E)  (one fused matmul with a ones-column on V)
        W = (1-2l)*V + 3l*diag(1/s)*U
        out_head = diag(1/s) * (E @ W)
    No softmax max-subtraction needed: logits ~ N(0,1) by construction.

Sharding: pure data-parallel over batch, 2 batches per core, no collectives.
All layout transposes are done host-side in numpy (x^T, W^T fed directly).
"""

import os

import numpy as np

B, N, C, H, HD = 16, 577, 768, 12, 64
NCORES = 8
BPC = B // NCORES  # batches per core
SCALE = HD**-0.5
PAIRS = H // 2
CT = C // 128  # 6 c-tiles

# token tiles (start, rows)
TOK = [(t * 128, min(128, N - t * 128)) for t in range((N + 127) // 128)]
NP = N + 1  # 578: fp32r matmuls need an even moving free dim, so pad tokens
ETC = [(0, 512), (512, NP - 512)]  # ET matmul free chunks (bank-aligned, even)
XQC = [(0, 290), (290, NP - 290)]  # q/k projection free chunks (>=256, even)
PC = [(0, 384), (384, 384)]  # v / proj free chunks

MODE = os.environ.get("KERNEL_MODE", "mixed")

_CACHE = {}


def _dtypes(mode):
    import concourse.mybir as mybir

    f32, f32r, bf16 = mybir.dt.float32, mybir.dt.float32r, mybir.dt.bfloat16
    if mode == "bf16":
        return dict(W=bf16, X=bf16, QK=bf16, ATT=bf16, TP=bf16, PROJ=bf16)
    if mode == "mixed":
        return dict(W=f32r, X=f32r, QK=f32r, ATT=bf16, TP=bf16, PROJ=bf16)
    if mode == "fp32r":
        return dict(W=f32r, X=f32r, QK=f32r, ATT=f32r, TP=f32, PROJ=f32r)
    if mode == "fp32":
        return dict(W=f32, X=f32, QK=f32, ATT=f32, TP=f32, PROJ=f32)
    raise ValueError(mode)


def build(mode=MODE, loop_n=1):
    """Build + compile the per-core Bass program (SPMD, identical on all cores)."""
    import concourse.mybir as mybir
    import concourse.tile as tile
    from concourse import bacc
    from concourse.masks import make_identity

    D = _dtypes(mode)
    f32 = mybir.dt.float32
    Exp = mybir.ActivationFunctionType.Exp
    mult, add = mybir.AluOpType.mult, mybir.AluOpType.add

    def cast(ap):
        return ap

    nc = bacc.Bacc("TRN2", target_bir_lowering=False, debug=False, num_devices=NCORES)

    xT = nc.dram_tensor("xT", [BPC, C, N], D["X"], kind="ExternalInput")
    wqT = nc.dram_tensor("wqT", [C, C], D["W"], kind="ExternalInput")
    wkT = nc.dram_tensor("wkT", [C, C], D["W"], kind="ExternalInput")
    wvT = nc.dram_tensor("wvT", [C, C], D["W"], kind="ExternalInput")
    wpT = nc.dram_tensor("wpT", [C, C], D["PROJ"], kind="ExternalInput")
    bpr = nc.dram_tensor("bpr", [1, C], D["PROJ"], kind="ExternalInput")
    cst = nc.dram_tensor("cst", [128, 2, H], f32, kind="ExternalInput")
    onesH = nc.dram_tensor("onesH", [128, H], D["ATT"], kind="ExternalInput")
    ones1 = nc.dram_tensor("ones1", [1, 128], D["PROJ"], kind="ExternalInput")
    out = nc.dram_tensor("out", [BPC, N, C], f32, kind="ExternalOutput")

    with tile.TileContext(nc) as tc:
        # per-mode buffer depths (SBUF budget: fp32 tiles are 2x bigger)
        big = mode == "bf16"
        xb = 2 if big else 1
        qb = 2 if big else 1
        vb = 2 if mode in ("bf16", "mixed") else 1
        eb = 2 if mode in ("bf16", "mixed") else 1
        zb = 2 if mode in ("bf16", "mixed") else 1
        with (
            tc.tile_pool(name="wp", bufs=1) as wpool,
            tc.tile_pool(name="cp", bufs=1) as cpool,
            tc.tile_pool(name="xp", bufs=xb) as xpool,
            tc.tile_pool(name="qkp", bufs=qb) as qkpool,
            tc.tile_pool(name="vap", bufs=vb) as vapool,
            tc.tile_pool(name="etp", bufs=eb) as etpool,
            tc.tile_pool(name="hsp", bufs=2) as hspool,
            tc.tile_pool(name="zcp", bufs=zb) as zcpool,
            tc.tile_pool(name="obp", bufs=3) as obpool,
            tc.tile_pool(name="ps_et", bufs=1, space="PSUM") as ps_et,
            tc.tile_pool(name="ps_sm", bufs=2, space="PSUM") as ps_sm,
            tc.tile_pool(name="ps_io", bufs=2, space="PSUM") as ps_io,
        ):
            # ---- persistent constants / weights ----
            wq = [wpool.tile([128, C], D["W"], tag=f"wq{ct}", name=f"wq{ct}") for ct in range(CT)]
            wk = [wpool.tile([128, C], D["W"], tag=f"wk{ct}", name=f"wk{ct}") for ct in range(CT)]
            wv = [wpool.tile([128, C], D["W"], tag=f"wv{ct}", name=f"wv{ct}") for ct in range(CT)]
            wp = [wpool.tile([128, C], D["PROJ"], tag=f"wp{ct}", name=f"wp{ct}") for ct in range(CT)]
            for ct in range(CT):
                sl = slice(ct * 128, (ct + 1) * 128)
                nc.sync.dma_start(wq[ct][:], wqT[sl, :])
                nc.sync.dma_start(wk[ct][:], wkT[sl, :])
                nc.sync.dma_start(wv[ct][:], wvT[sl, :])
                nc.sync.dma_start(wp[ct][:], wpT[sl, :])
            cst_sb = cpool.tile([128, 2, H], f32, tag="cst", name="cst_sb")
            nc.sync.dma_start(cst_sb[:], cst[:, :, :])
            bp_sb = cpool.tile([1, C], D["PROJ"], tag="bp", name="bp_sb")
            nc.sync.dma_start(bp_sb[:], bpr[:, :])
            ones_row = cpool.tile([1, 128], D["PROJ"], tag="ones1", name="ones_row")
            nc.sync.dma_start(ones_row[:], ones1[:, :])
            ident = cpool.tile([128, 128], D["TP"], tag="id", name="ident")
            make_identity(nc, ident[:])

            def body():
                for b in range(BPC):
                    # ---- load x^T ----
                    xt = []
                    for ct in range(CT):
                        t_ = xpool.tile([128, NP], D["X"], tag=f"xt{ct}", name=f"xt{ct}")
                        nc.sync.dma_start(t_[:, :N], xT[b, ct * 128 : (ct + 1) * 128, :])
                        xt.append(t_)
                    # ---- q^T, k^T  ([d,n] layout, head pair per 128-tile) ----
                    qt, kt = [], []
                    for name, w, dst in (("q", wq, qt), ("k", wk, kt)):
                        for dtt in range(CT):
                            o = qkpool.tile([128, NP], D["QK"], tag=f"{name}{dtt}", name=f"{name}t{dtt}")
                            for c0, cl in XQC:
                                ps = ps_io.tile([128, 512], f32, tag="io", name="iops")
                                for ct in range(CT):
                                    nc.tensor.matmul(
                                        ps[:, :cl],
                                        lhsT=cast(w[ct][:, dtt * 128 : dtt * 128 + 128]),
                                        rhs=cast(xt[ct][:, c0 : c0 + cl]),
                                        start=(ct == 0),
                                        stop=(ct == CT - 1),
                                    )
                                nc.scalar.copy(o[:, c0 : c0 + cl], ps[:, :cl])
                            dst.append(o)
                    # ---- V (normal [n, head*hd] layout + ones column per head) ----
                    va = []
                    for ti, (t0, rows) in enumerate(TOK):
                        t_ = vapool.tile([128, H, HD + 2], D["ATT"], tag=f"va{ti}", name=f"va{ti}")
                        nc.sync.dma_start(t_[:rows, :, HD : HD + 1], onesH[:rows, :])
                        for half, (m0, ml) in enumerate(PC):
                            ps = ps_io.tile([128, 512], f32, tag="io", name="iops")
                            for ct in range(CT):
                                nc.tensor.matmul(
                                    ps[:rows, :ml],
                                    lhsT=cast(xt[ct][:, t0 : t0 + rows]),
                                    rhs=cast(wv[ct][:, m0 : m0 + ml]),
                                    start=(ct == 0),
                                    stop=(ct == CT - 1),
                                )
                            nc.vector.tensor_copy(
                                t_[:rows, 6 * half : 6 * half + 6, :HD],
                                ps[:rows, :ml].rearrange("p (h d) -> p h d", d=HD),
                            )
                        va.append(t_)
                    # ---- head pairs ----
                    zc = [
                        zcpool.tile([128, N], D["PROJ"], tag=f"zc{ct}", name=f"zc{ct}")
                        for ct in range(CT)
                    ]
                    for p in range(PAIRS):
                        # E^T = exp(scale * K Q^T) per head of the pair
                        ets = []
                        for jt, (j0, jrows) in enumerate(TOK):
                            eps = ps_et.tile([128, 2, 1024], f32, tag="et", name="eps")
                            for h01 in range(2):
                                lo = 64 * h01
                                for i0, il in ETC:
                                    nc.tensor.matmul(
                                        eps[:jrows, h01, i0 : i0 + il],
                                        lhsT=cast(kt[p][lo : lo + 64, j0 : j0 + jrows]),
                                        rhs=cast(qt[p][lo : lo + 64, i0 : i0 + il]),
                                        start=True,
                                        stop=True,
                                    )
                            esb = etpool.tile([128, 2, NP], D["ATT"], tag=f"et{jt}", name=f"esb{jt}")
                            nc.scalar.activation(
                                esb[:jrows, :, :], eps[:jrows, :, :NP], Exp, scale=SCALE
                            )
                            ets.append(esb)
                        ohs = [
                            hspool.tile([128, 128], D["TP"], tag=f"oh{it}", name=f"oh{it}")
                            for it in range(len(TOK))
                        ]
                        for h01 in range(2):
                            head = 2 * p + h01
                            c1a = cst_sb[:, 0, head : head + 1]
                            c3a = cst_sb[:, 1, head : head + 1]
                            ws, srs = [], []
                            # U_aug = E @ [V | 1]  -> U, s ; then W = c1*V + c3*(U/s)
                            for it, (t0, rows) in enumerate(TOK):
                                ups = ps_sm.tile([128, 128], f32, tag="sm", name="smps")
                                for jt, (j0, jrows) in enumerate(TOK):
                                    nc.tensor.matmul(
                                        ups[:rows, : HD + 2],
                                        lhsT=cast(ets[jt][:jrows, h01, t0 : t0 + rows]),
                                        rhs=cast(va[jt][:jrows, head, :]),
                                        start=(jt == 0),
                                        stop=(jt == len(TOK) - 1),
                                    )
                                sr = hspool.tile([128, 1], f32, tag=f"sr{h01}_{it}", name=f"sr{h01}_{it}")
                                nc.vector.reciprocal(sr[:rows], ups[:rows, HD : HD + 1])
                                pp = hspool.tile([128, HD], D["ATT"], tag="pp", name="pp")
                                nc.vector.tensor_scalar(
                                    pp[:rows],
                                    ups[:rows, :HD],
                                    sr[:rows],
                                    c3a[:rows],
                                    op0=mult,
                                    op1=mult,
                                )
                                w_ = hspool.tile([128, HD], D["ATT"], tag=f"wj{it}", name=f"wj{it}")
                                nc.vector.scalar_tensor_tensor(
                                    w_[:rows],
                                    va[it][:rows, head, :HD],
                                    c1a[:rows],
                                    pp[:rows],
                                    op0=mult,
                                    op1=add,
                                )
                                ws.append(w_)
                                srs.append(sr)
                            # Z = E @ W ; out_head = Z / s
                            for it, (t0, rows) in enumerate(TOK):
                                zps = ps_sm.tile([128, 128], f32, tag="sm", name="smps")
                                for jt, (j0, jrows) in enumerate(TOK):
                                    nc.tensor.matmul(
                                        zps[:rows, :HD],
                                        lhsT=cast(ets[jt][:jrows, h01, t0 : t0 + rows]),
                                        rhs=cast(ws[jt][:jrows, :]),
                                        start=(jt == 0),
                                        stop=(jt == len(TOK) - 1),
                                    )
                                nc.vector.tensor_scalar_mul(
                                    ohs[it][:rows, 64 * h01 : 64 * h01 + HD],
                                    zps[:rows, :HD],
                                    srs[it][:rows],
                                )
                        # transpose pair -> zcat^T rows
                        for it, (t0, rows) in enumerate(TOK):
                            tps = ps_sm.tile([128, 128], D["TP"], tag="sm", name="smps")
                            nc.tensor.transpose(
                                tps[:, :rows], ohs[it][:rows, :], ident[:rows, :rows]
                            )
                            nc.vector.tensor_copy(zc[p][:, t0 : t0 + rows], tps[:, :rows])
                    # ---- projection + bias, then store ----
                    for it, (t0, rows) in enumerate(TOK):
                        ob = obpool.tile([128, C], f32, tag="ob", name="ob")
                        for half, (m0, ml) in enumerate(PC):
                            pps = ps_io.tile([128, 512], f32, tag="io", name="iops")
                            for ct in range(CT):
                                nc.tensor.matmul(
                                    pps[:rows, :ml],
                                    lhsT=cast(zc[ct][:, t0 : t0 + rows]),
                                    rhs=cast(wp[ct][:, m0 : m0 + ml]),
                                    start=(ct == 0),
                                    stop=False,
                                )
                            nc.tensor.matmul(
                                pps[:rows, :ml],
                                lhsT=cast(ones_row[:, :rows]),
                                rhs=cast(bp_sb[:, m0 : m0 + ml]),
                                start=False,
                                stop=True,
                            )
                            nc.scalar.copy(ob[:rows, m0 : m0 + ml], pps[:rows, :ml])
                        nc.sync.dma_start(out[b, t0 : t0 + rows, :], ob[:rows, :])

            if loop_n > 1:
                with tc.For_i(0, loop_n, 1):
                    body()
            else:
                body()

    nc.compile()
    return nc


def _prep_in_maps(mode, x, Wq, Wk, Wv, Wp, bp, lamb):
    import ml_dtypes

    D = _dtypes(mode)
    bf16 = ml_dtypes.bfloat16

    def npdt(d):
        import concourse.mybir as mybir

        return bf16 if d == mybir.dt.bfloat16 else np.float32

    npW, npX, npPROJ = npdt(D["W"]), npdt(D["X"]), npdt(D["PROJ"])
    wqT = np.ascontiguousarray(Wq.T).astype(npW)
    wkT = np.ascontiguousarray(Wk.T).astype(npW)
    wvT = np.ascontiguousarray(Wv.T).astype(npW)
    wpT = np.ascontiguousarray(Wp.T).astype(npPROJ)
    bpr = np.ascontiguousarray(bp.reshape(1, C)).astype(npPROJ)
    c1 = (1.0 - 2.0 * lamb).astype(np.float32)
    c3 = (3.0 * lamb).astype(np.float32)
    cstv = np.ascontiguousarray(
        np.broadcast_to(np.stack([c1, c3], 0)[None], (128, 2, H))
    ).astype(np.float32)
    onesHv = np.ones((128, H), dtype=npdt(D["ATT"]))
    ones1v = np.ones((1, 128), dtype=npPROJ)
    in_maps = []
    for core in range(NCORES):
        xs = x[core * BPC : (core + 1) * BPC]
        xTv = np.ascontiguousarray(xs.transpose(0, 2, 1)).astype(npX)
        in_maps.append(
            dict(xT=xTv, wqT=wqT, wkT=wkT, wvT=wvT, wpT=wpT, bpr=bpr, cst=cstv,
                 onesH=onesHv, ones1=ones1v)
        )
    return in_maps


def kernel(x, Wq, Wk, Wv, Wp, bp, lamb):
    from concourse.bass_utils import run_bass_kernel_spmd

    x = np.asarray(x, dtype=np.float32)
    Wq = np.asarray(Wq, dtype=np.float32)
    Wk = np.asarray(Wk, dtype=np.float32)
    Wv = np.asarray(Wv, dtype=np.float32)
    Wp = np.asarray(Wp, dtype=np.float32)
    bp = np.asarray(bp, dtype=np.float32)
    lamb = np.asarray(lamb, dtype=np.float32)

    if MODE not in _CACHE:
        _CACHE[MODE] = build(MODE)
    nc = _CACHE[MODE]
    in_maps = _prep_in_maps(MODE, x, Wq, Wk, Wv, Wp, bp, lamb)
    res = run_bass_kernel_spmd(nc, in_maps, list(range(NCORES)))
    return np.concatenate([res.results[i]["out"] for i in range(NCORES)], axis=0)


# revision 18
# speedup vs baseline: 9.5379x; 9.5379x over previous
"""Class-attention with GFSA reaction term — Trainium2 Bass kernel, 8 NeuronCores.

Math (reference):
    q,k,v = x@W{q,k,v}.T split into H=12 heads of 64
    A  = softmax(q k^T / 8)                  per (b,h), [N,N], N=577
    attn = A + lamb_h * (3*A@A - 2*A)
    out  = (attn @ v) @ Wp.T + bp

Key algebraic restructure (avoids the N^3 A@A entirely):
    out_head = (1-2l)*A@V + 3l*A@(A@V)
    With E = exp(logits) unnormalized and D = diag(rowsum(E)):
        U = E@V,  s = rowsum(E)  (one fused matmul with a ones-column on V)
        W = (1-2l)*V + 3l*diag(1/s)*U
        out_head = diag(1/s) * (E @ W)
    No softmax max-subtraction needed: logits ~ N(0,1) by construction.

Sharding: pure data-parallel over batch, 2 batches per core, no collectives.
All layout transposes are done host-side in numpy (x^T, W^T fed directly).
"""

import os
import sys

import numpy as np

for _p in ("/opt/trn_rl_repo", "/root/.axon_site", "/root/.axon_site/_ro/trn_rl_repo"):
    if _p not in sys.path and os.path.isdir(_p):
        sys.path.append(_p)

B, N, C, H, HD = 16, 577, 768, 12, 64
NCORES = 8
BPC = B // NCORES  # batches per core
SCALE = HD**-0.5
PAIRS = H // 2
CT = C // 128  # 6 c-tiles

# token tiles (start, rows)
TOK = [(t * 128, min(128, N - t * 128)) for t in range((N + 127) // 128)]
NP = N + 1  # 578: fp32r matmuls need an even moving free dim, so pad tokens
ETC = [(0, 512), (512, NP - 512)]  # ET matmul free chunks (bank-aligned, even)
XQC = [(0, 512), (512, NP - 512)]  # q/k projection free chunks (even)
PC = [(0, 384), (384, 384)]  # v / proj free chunks

MODE = os.environ.get("KERNEL_MODE", "bf16")

_CACHE = {}


def _dtypes(mode):
    import concourse.mybir as mybir

    f32, f32r, bf16 = mybir.dt.float32, mybir.dt.float32r, mybir.dt.bfloat16
    if mode == "bf16":
        return dict(W=bf16, X=bf16, QK=bf16, ATT=bf16, TP=bf16, PROJ=bf16)
    if mode == "mixed":
        return dict(W=f32r, X=f32r, QK=f32r, ATT=bf16, TP=bf16, PROJ=bf16)
    if mode == "fp32r":
        return dict(W=f32r, X=f32r, QK=f32r, ATT=f32r, TP=f32, PROJ=f32r)
    if mode == "fp32":
        return dict(W=f32, X=f32, QK=f32, ATT=f32, TP=f32, PROJ=f32)
    raise ValueError(mode)


def build(mode=MODE, loop_n=1, ablate=(), probe=()):
    """Build + compile the per-core Bass program (SPMD, identical on all cores)."""
    import concourse.mybir as mybir
    import concourse.tile as tile
    from concourse import bacc
    from concourse.masks import make_identity

    D = _dtypes(mode)
    f32 = mybir.dt.float32
    Exp = mybir.ActivationFunctionType.Exp
    mult, add = mybir.AluOpType.mult, mybir.AluOpType.add

    def cast(ap):
        return ap

    nc = bacc.Bacc("TRN2", target_bir_lowering=False, debug=False, num_devices=NCORES)

    xT = nc.dram_tensor("xT", [BPC, C, N], D["X"], kind="ExternalInput")
    wqT = nc.dram_tensor("wqT", [C, C], D["W"], kind="ExternalInput")
    wkT = nc.dram_tensor("wkT", [C, C], D["W"], kind="ExternalInput")
    wvT = nc.dram_tensor("wvT", [C, C], D["W"], kind="ExternalInput")
    wpT = nc.dram_tensor("wpT", [C, C], D["PROJ"], kind="ExternalInput")
    bpr = nc.dram_tensor("bpr", [1, C], D["PROJ"], kind="ExternalInput")
    cst = nc.dram_tensor("cst", [128, 2, H], f32, kind="ExternalInput")
    onesH = nc.dram_tensor("onesH", [128, H], D["ATT"], kind="ExternalInput")
    ones1 = nc.dram_tensor("ones1", [1, 128], D["PROJ"], kind="ExternalInput")
    out = nc.dram_tensor("out", [BPC, N, C], f32, kind="ExternalOutput")

    with tile.TileContext(nc) as tc:
        # per-mode buffer depths (SBUF budget: fp32 tiles are 2x bigger)
        big = mode == "bf16"
        xb = 2 if big else 1
        qb = 2 if big else 1
        vb = 2 if mode in ("bf16", "mixed") else 1
        eb = 2 if mode in ("bf16", "mixed") else 1
        zb = 2 if mode in ("bf16", "mixed") else 1
        with (
            tc.tile_pool(name="wp", bufs=1) as wpool,
            tc.tile_pool(name="cp", bufs=1) as cpool,
            tc.tile_pool(name="xp", bufs=xb) as xpool,
            tc.tile_pool(name="qkp", bufs=qb) as qkpool,
            tc.tile_pool(name="vap", bufs=vb) as vapool,
            tc.tile_pool(name="etp", bufs=eb + 1) as etpool,
            tc.tile_pool(name="hsp", bufs=3) as hspool,
            tc.tile_pool(name="zcp", bufs=zb) as zcpool,
            tc.tile_pool(name="obp", bufs=3) as obpool,
            tc.tile_pool(name="ps_et", bufs=2, space="PSUM") as ps_et,
            tc.tile_pool(name="ps_u", bufs=1, space="PSUM") as ps_u,
            tc.tile_pool(name="ps_z", bufs=1, space="PSUM") as ps_z,
            tc.tile_pool(name="ps_io", bufs=2, space="PSUM") as ps_io,
        ):
            # ---- persistent constants / weights ----
            wq = [wpool.tile([128, C], D["W"], tag=f"wq{ct}", name=f"wq{ct}") for ct in range(CT)]
            wk = [wpool.tile([128, C], D["W"], tag=f"wk{ct}", name=f"wk{ct}") for ct in range(CT)]
            wv = [wpool.tile([128, C], D["W"], tag=f"wv{ct}", name=f"wv{ct}") for ct in range(CT)]
            wp = [wpool.tile([128, C], D["PROJ"], tag=f"wp{ct}", name=f"wp{ct}") for ct in range(CT)]
            # weights on the scalar HWDGE queue so they stream in parallel
            # with the x^T loads issued on the sync queue inside body()
            for ct in range(CT):
                sl = slice(ct * 128, (ct + 1) * 128)
                nc.scalar.dma_start(wq[ct][:], wqT[sl, :])
                nc.scalar.dma_start(wk[ct][:], wkT[sl, :])
            for ct in range(CT):
                sl = slice(ct * 128, (ct + 1) * 128)
                nc.scalar.dma_start(wv[ct][:], wvT[sl, :])
                nc.scalar.dma_start(wp[ct][:], wpT[sl, :])
            cst_sb = cpool.tile([128, 2, H], f32, tag="cst", name="cst_sb")
            nc.sync.dma_start(cst_sb[:], cst[:, :, :])
            bp_sb = cpool.tile([1, C], D["PROJ"], tag="bp", name="bp_sb")
            nc.sync.dma_start(bp_sb[:], bpr[:, :])
            ones_row = cpool.tile([1, 128], D["PROJ"], tag="ones1", name="ones_row")
            nc.sync.dma_start(ones_row[:], ones1[:, :])
            ident = cpool.tile([128, 128], D["TP"], tag="id", name="ident")
            make_identity(nc, ident[:])

            def body():
                for b in range(BPC):
                    # ---- load x^T ----
                    xt = []
                    for ct in range(CT):
                        t_ = xpool.tile([128, NP], D["X"], tag=f"xt{ct}", name=f"xt{ct}")
                        nc.sync.dma_start(t_[:, :N], xT[b, ct * 128 : (ct + 1) * 128, :])
                        xt.append(t_)
                    # ---- q^T, k^T  ([d,n] layout, head pair per 128-tile) ----
                    qt, kt = [], []
                    for name, w, dst in (("q", wq, qt), ("k", wk, kt)) if "qkv" not in ablate else ():
                        for dtt in range(CT):
                            o = qkpool.tile([128, NP], D["QK"], tag=f"{name}{dtt}", name=f"{name}t{dtt}")
                            for c0, cl in XQC:
                                ps = ps_io.tile([128, 512], f32, tag="io", name="iops")
                                for ct in range(CT):
                                    nc.tensor.matmul(
                                        ps[:, :cl],
                                        lhsT=cast(w[ct][:, dtt * 128 : dtt * 128 + 128]),
                                        rhs=cast(xt[ct][:, c0 : c0 + cl]),
                                        start=(ct == 0),
                                        stop=(ct == CT - 1),
                                    )
                                nc.vector.tensor_copy(o[:, c0 : c0 + cl], ps[:, :cl])
                            dst.append(o)
                    # ---- V (normal [n, head*hd] layout + ones column per head) ----
                    va = []
                    for ti, (t0, rows) in enumerate(TOK):
                        if "qkv" in ablate:
                            t_ = vapool.tile([128, H, HD + 2], D["ATT"], tag=f"va{ti}", name=f"va{ti}")
                            va.append(t_)
                            continue
                        t_ = vapool.tile([128, H, HD + 2], D["ATT"], tag=f"va{ti}", name=f"va{ti}")
                        nc.sync.dma_start(t_[:rows, :, HD : HD + 1], onesH[:rows, :])
                        for half, (m0, ml) in enumerate(PC):
                            ps = ps_io.tile([128, 512], f32, tag="io", name="iops")
                            for ct in range(CT):
                                nc.tensor.matmul(
                                    ps[:rows, :ml],
                                    lhsT=cast(xt[ct][:, t0 : t0 + rows]),
                                    rhs=cast(wv[ct][:, m0 : m0 + ml]),
                                    start=(ct == 0),
                                    stop=(ct == CT - 1),
                                )
                            nc.scalar.copy(
                                t_[:rows, 6 * half : 6 * half + 6, :HD],
                                ps[:rows, :ml].rearrange("p (h d) -> p h d", d=HD),
                            )
                        va.append(t_)
                    # ---- head pairs ----
                    zc = [
                        zcpool.tile([128, N], D["PROJ"], tag=f"zc{ct}", name=f"zc{ct}")
                        for ct in range(CT)
                    ]
                    for p in range(PAIRS):
                        if "pairs" in ablate:
                            break
                        # E^T = exp(scale * K Q^T), per head of the pair.
                        # Separate 2-bank psum tiles per (jt, head) with
                        # bufs=2 so head B's logits matmuls overlap head A's
                        # exp on ACT.
                        ets = []
                        for jt, (j0, jrows) in enumerate(TOK):
                            epair = []
                            for h01 in range(2):
                                lo = 64 * h01
                                eps = ps_et.tile([128, 1024], f32, tag="et", name="eps")
                                for i0, il in ETC:
                                    nc.tensor.matmul(
                                        eps[:jrows, i0 : i0 + il],
                                        lhsT=cast(kt[p][lo : lo + 64, j0 : j0 + jrows]),
                                        rhs=cast(qt[p][lo : lo + 64, i0 : i0 + il]),
                                        start=True,
                                        stop=True,
                                    )
                                esb = etpool.tile(
                                    [128, NP], D["ATT"], tag=f"et{jt}_{h01}", name=f"esb{jt}_{h01}"
                                )
                                nc.scalar.activation(
                                    esb[:jrows, :], eps[:jrows, :NP], Exp, scale=SCALE
                                )
                                epair.append(esb)
                            ets.append(epair)
                        ohs = [
                            hspool.tile([128, 128], D["TP"], tag=f"oh{it}", name=f"oh{it}")
                            for it in range(len(TOK))
                        ]
                        for h01 in range(2 * ("uz" not in ablate)):
                            head = 2 * p + h01
                            c1a = cst_sb[:, 0, head : head + 1]
                            c3a = cst_sb[:, 1, head : head + 1]
                            nit = len(TOK)
                            # U_aug = E @ [V | 1] -> U, s. Round-major (jt outer)
                            # so each round's matmuls run as soon as exp(jt) is
                            # done; all 5 token tiles accumulate in ONE psum
                            # bank ([128, 5, 66]).
                            ups = ps_u.tile([128, nit, HD + 2], f32, tag="u", name="ups")
                            ujts = [0, 2, 4] if "skipjt" in probe else range(len(TOK))
                            for jt in ujts:
                                (j0, jrows) = TOK[jt]
                                for it, (t0, rows) in enumerate(TOK):
                                    nc.tensor.matmul(
                                        ups[:rows, it, : (2 if "tinyuz" in probe else HD + 2)],
                                        lhsT=cast(ets[jt][h01][:jrows, t0 : t0 + rows]),
                                        rhs=cast(va[jt][:jrows, head, : (2 if "tinyuz" in probe else HD + 2)]),
                                        start=(jt == ujts[0] and it == 0),
                                        stop=(jt == ujts[-1]),
                                    )
                            ws, srs = [], []
                            for it, (t0, rows) in enumerate(TOK):
                                sr = hspool.tile([128, 1], f32, tag=f"sr{h01}_{it}", name=f"sr{h01}_{it}")
                                nc.vector.reciprocal(sr[:rows], va[it][:rows, head, :1] if "detach" in probe else ups[:rows, it, HD : HD + 1])
                                pp = hspool.tile([128, HD], D["ATT"], tag="pp", name="pp")
                                nc.vector.tensor_scalar(
                                    pp[:rows],
                                    va[it][:rows, head, :HD] if "detach" in probe else ups[:rows, it, :HD],
                                    sr[:rows],
                                    c3a[:rows],
                                    op0=mult,
                                    op1=mult,
                                )
                                w_ = hspool.tile([128, HD], D["ATT"], tag=f"wj{it}", name=f"wj{it}")
                                nc.vector.scalar_tensor_tensor(
                                    w_[:rows],
                                    va[it][:rows, head, :HD],
                                    c1a[:rows],
                                    pp[:rows],
                                    op0=mult,
                                    op1=add,
                                )
                                ws.append(w_)
                                srs.append(sr)
                            if "z" in ablate:
                                continue
                            # Z = E @ W ; out_head = Z / s
                            zps = ps_z.tile([128, nit, HD], f32, tag="z", name="zps")
                            for jt in ujts:
                                (j0, jrows) = TOK[jt]
                                for it, (t0, rows) in enumerate(TOK):
                                    nc.tensor.matmul(
                                        zps[:rows, it, : (2 if "tinyuz" in probe else HD)],
                                        lhsT=cast(ets[jt][h01][:jrows, t0 : t0 + rows]),
                                        rhs=cast(ws[jt][:jrows, : (2 if "tinyuz" in probe else HD)]),
                                        start=(jt == ujts[0] and it == 0),
                                        stop=(jt == ujts[-1]),
                                    )
                            for it, (t0, rows) in enumerate(TOK):
                                nc.vector.tensor_scalar_mul(
                                    ohs[it][:rows, 64 * h01 : 64 * h01 + HD],
                                    va[it][:rows, head, :HD] if "detach" in probe else zps[:rows, it, :],
                                    srs[it][:rows],
                                )
                        # transpose pair -> zcat^T rows
                        for it, (t0, rows) in enumerate(TOK):
                            if "tp" in ablate:
                                break
                            tps = ps_io.tile([128, 128], D["TP"], tag="io", name="tps")
                            nc.tensor.transpose(
                                tps[:, :rows], ohs[it][:rows, :], ident[:rows, :rows]
                            )
                            nc.vector.tensor_copy(zc[p][:, t0 : t0 + rows], tps[:, :rows])
                    # ---- projection + bias, then store ----
                    for it, (t0, rows) in enumerate(TOK):
                        if "proj" in ablate:
                            break
                        ob = obpool.tile([128, C], f32, tag="ob", name="ob")
                        for half, (m0, ml) in enumerate(PC):
                            pps = ps_io.tile([128, 512], f32, tag="io", name="iops")
                            for ct in range(CT):
                                nc.tensor.matmul(
                                    pps[:rows, :ml],
                                    lhsT=cast(zc[ct][:, t0 : t0 + rows]),
                                    rhs=cast(wp[ct][:, m0 : m0 + ml]),
                                    start=(ct == 0),
                                    stop=False,
                                )
                            nc.tensor.matmul(
                                pps[:rows, :ml],
                                lhsT=cast(ones_row[:, :rows]),
                                rhs=cast(bp_sb[:, m0 : m0 + ml]),
                                start=False,
                                stop=True,
                            )
                            nc.scalar.copy(ob[:rows, m0 : m0 + ml], pps[:rows, :ml])
                        nc.sync.dma_start(out[b, t0 : t0 + rows, :], ob[:rows, :])

            if loop_n > 1:
                with tc.For_i(0, loop_n, 1):
                    body()
            else:
                body()

    nc.compile()
    return nc


def _prep_in_maps(mode, x, Wq, Wk, Wv, Wp, bp, lamb):
    import ml_dtypes

    D = _dtypes(mode)
    bf16 = ml_dtypes.bfloat16

    def npdt(d):
        import concourse.mybir as mybir

        return bf16 if d == mybir.dt.bfloat16 else np.float32

    npW, npX, npPROJ = npdt(D["W"]), npdt(D["X"]), npdt(D["PROJ"])
    wqT = np.ascontiguousarray(Wq.T).astype(npW)
    wkT = np.ascontiguousarray(Wk.T).astype(npW)
    wvT = np.ascontiguousarray(Wv.T).astype(npW)
    wpT = np.ascontiguousarray(Wp.T).astype(npPROJ)
    bpr = np.ascontiguousarray(bp.reshape(1, C)).astype(npPROJ)
    c1 = (1.0 - 2.0 * lamb).astype(np.float32)
    c3 = (3.0 * lamb).astype(np.float32)
    cstv = np.ascontiguousarray(
        np.broadcast_to(np.stack([c1, c3], 0)[None], (128, 2, H))
    ).astype(np.float32)
    onesHv = np.ones((128, H), dtype=npdt(D["ATT"]))
    ones1v = np.ones((1, 128), dtype=npPROJ)
    in_maps = []
    for core in range(NCORES):
        xs = x[core * BPC : (core + 1) * BPC]
        xTv = np.ascontiguousarray(xs.transpose(0, 2, 1)).astype(npX)
        in_maps.append(
            dict(xT=xTv, wqT=wqT, wkT=wkT, wvT=wvT, wpT=wpT, bpr=bpr, cst=cstv,
                 onesH=onesHv, ones1=ones1v)
        )
    return in_maps


def kernel(x, Wq, Wk, Wv, Wp, bp, lamb):
    from concourse.bass_utils import run_bass_kernel_spmd

    x = np.asarray(x, dtype=np.float32)
    Wq = np.asarray(Wq, dtype=np.float32)
    Wk = np.asarray(Wk, dtype=np.float32)
    Wv = np.asarray(Wv, dtype=np.float32)
    Wp = np.asarray(Wp, dtype=np.float32)
    bp = np.asarray(bp, dtype=np.float32)
    lamb = np.asarray(lamb, dtype=np.float32)

    if MODE not in _CACHE:
        _CACHE[MODE] = build(MODE)
    nc = _CACHE[MODE]
    in_maps = _prep_in_maps(MODE, x, Wq, Wk, Wv, Wp, bp, lamb)
    res = run_bass_kernel_spmd(nc, in_maps, list(range(NCORES)))
    return np.concatenate([res.results[i]["out"] for i in range(NCORES)], axis=0)


# revision 21
# speedup vs baseline: 9.7692x; 1.0243x over previous
"""Class-attention with GFSA reaction term — Trainium2 Bass kernel, 8 NeuronCores.

Math (reference):
    q,k,v = x@W{q,k,v}.T split into H=12 heads of 64
    A  = softmax(q k^T / 8)                  per (b,h), [N,N], N=577
    attn = A + lamb_h * (3*A@A - 2*A)
    out  = (attn @ v) @ Wp.T + bp

Key algebraic restructure (avoids the N^3 A@A entirely):
    out_head = (1-2l)*A@V + 3l*A@(A@V)
    With E = exp(logits) unnormalized and D = diag(rowsum(E)):
        U = E@V,  s = rowsum(E)  (one fused matmul with a ones-column on V)
        W = (1-2l)*V + 3l*diag(1/s)*U
        out_head = diag(1/s) * (E @ W)
    No softmax max-subtraction needed: logits ~ N(0,1) by construction.

Sharding: pure data-parallel over batch, 2 batches per core, no collectives.
All layout transposes are done host-side in numpy (x^T, W^T fed directly).
"""

import os
import sys

import numpy as np

for _p in ("/opt/trn_rl_repo", "/root/.axon_site", "/root/.axon_site/_ro/trn_rl_repo"):
    if _p not in sys.path and os.path.isdir(_p):
        sys.path.append(_p)

B, N, C, H, HD = 16, 577, 768, 12, 64
NCORES = 8
BPC = B // NCORES  # batches per core
SCALE = HD**-0.5
PAIRS = H // 2
CT = C // 128  # 6 c-tiles

# token tiles (start, rows)
TOK = [(t * 128, min(128, N - t * 128)) for t in range((N + 127) // 128)]
NP = N + 1  # 578: fp32r matmuls need an even moving free dim, so pad tokens
ETC = [(0, 512), (512, NP - 512)]  # ET matmul free chunks (bank-aligned, even)
XQC = [(0, 512), (512, NP - 512)]  # q/k projection free chunks (even)
PC = [(0, 384), (384, 384)]  # v / proj free chunks

MODE = os.environ.get("KERNEL_MODE", "bf16")

_CACHE = {}


def _dtypes(mode):
    import concourse.mybir as mybir

    f32, f32r, bf16 = mybir.dt.float32, mybir.dt.float32r, mybir.dt.bfloat16
    if mode == "bf16":
        return dict(W=bf16, X=bf16, QK=bf16, ATT=bf16, TP=bf16, PROJ=bf16)
    if mode == "mixed":
        return dict(W=f32r, X=f32r, QK=f32r, ATT=bf16, TP=bf16, PROJ=bf16)
    if mode == "fp32r":
        return dict(W=f32r, X=f32r, QK=f32r, ATT=f32r, TP=f32, PROJ=f32r)
    if mode == "fp32":
        return dict(W=f32, X=f32, QK=f32, ATT=f32, TP=f32, PROJ=f32)
    raise ValueError(mode)


def build(mode=MODE, loop_n=1, ablate=(), probe=()):
    """Build + compile the per-core Bass program (SPMD, identical on all cores)."""
    import concourse.mybir as mybir
    import concourse.tile as tile
    from concourse import bacc
    from concourse.masks import make_identity

    D = _dtypes(mode)
    f32 = mybir.dt.float32
    Exp = mybir.ActivationFunctionType.Exp
    mult, add = mybir.AluOpType.mult, mybir.AluOpType.add

    def cast(ap):
        return ap

    nc = bacc.Bacc("TRN2", target_bir_lowering=False, debug=False, num_devices=NCORES)

    xT = nc.dram_tensor("xT", [BPC, C, N], D["X"], kind="ExternalInput")
    wqT = nc.dram_tensor("wqT", [C, C], D["W"], kind="ExternalInput")
    wkT = nc.dram_tensor("wkT", [C, C], D["W"], kind="ExternalInput")
    wvT = nc.dram_tensor("wvT", [C, C], D["W"], kind="ExternalInput")
    wpT = nc.dram_tensor("wpT", [C, C], D["PROJ"], kind="ExternalInput")
    bpr = nc.dram_tensor("bpr", [1, C], D["PROJ"], kind="ExternalInput")
    cst = nc.dram_tensor("cst", [128, 2, H], f32, kind="ExternalInput")
    onesH = nc.dram_tensor("onesH", [128, H], D["ATT"], kind="ExternalInput")
    ones1 = nc.dram_tensor("ones1", [1, 128], D["PROJ"], kind="ExternalInput")
    out = nc.dram_tensor("out", [BPC, N, C], f32, kind="ExternalOutput")

    with tile.TileContext(nc) as tc:
        # per-mode buffer depths (SBUF budget: fp32 tiles are 2x bigger)
        big = mode == "bf16"
        xb = 2 if big else 1
        qb = 2 if big else 1
        vb = 2 if mode in ("bf16", "mixed") else 1
        eb = 2 if mode in ("bf16", "mixed") else 1
        zb = 2 if mode in ("bf16", "mixed") else 1
        with (
            tc.tile_pool(name="wp", bufs=1) as wpool,
            tc.tile_pool(name="cp", bufs=1) as cpool,
            tc.tile_pool(name="xp", bufs=xb) as xpool,
            tc.tile_pool(name="qkp", bufs=qb) as qkpool,
            tc.tile_pool(name="vap", bufs=vb) as vapool,
            tc.tile_pool(name="etp", bufs=eb + 1) as etpool,
            tc.tile_pool(name="hsp", bufs=3) as hspool,
            tc.tile_pool(name="zcp", bufs=zb) as zcpool,
            tc.tile_pool(name="obp", bufs=3) as obpool,
            tc.tile_pool(name="ps_et", bufs=2, space="PSUM") as ps_et,
            tc.tile_pool(name="ps_u", bufs=1, space="PSUM") as ps_u,
            tc.tile_pool(name="ps_z", bufs=1, space="PSUM") as ps_z,
            tc.tile_pool(name="ps_io", bufs=2, space="PSUM") as ps_io,
        ):
            # ---- persistent constants / weights ----
            wq = [wpool.tile([128, C], D["W"], tag=f"wq{ct}", name=f"wq{ct}") for ct in range(CT)]
            wk = [wpool.tile([128, C], D["W"], tag=f"wk{ct}", name=f"wk{ct}") for ct in range(CT)]
            wv = [wpool.tile([128, C], D["W"], tag=f"wv{ct}", name=f"wv{ct}") for ct in range(CT)]
            wp = [wpool.tile([128, C], D["PROJ"], tag=f"wp{ct}", name=f"wp{ct}") for ct in range(CT)]
            # weights on the scalar HWDGE queue so they stream in parallel
            # with the x^T loads issued on the sync queue inside body()
            for ct in range(CT):
                sl = slice(ct * 128, (ct + 1) * 128)
                nc.scalar.dma_start(wq[ct][:], wqT[sl, :])
                nc.scalar.dma_start(wk[ct][:], wkT[sl, :])
            for ct in range(CT):
                sl = slice(ct * 128, (ct + 1) * 128)
                nc.scalar.dma_start(wv[ct][:], wvT[sl, :])
                nc.scalar.dma_start(wp[ct][:], wpT[sl, :])
            cst_sb = cpool.tile([128, 2, H], f32, tag="cst", name="cst_sb")
            nc.sync.dma_start(cst_sb[:], cst[:, :, :])
            bp_sb = cpool.tile([1, C], D["PROJ"], tag="bp", name="bp_sb")
            nc.sync.dma_start(bp_sb[:], bpr[:, :])
            ones_row = cpool.tile([1, 128], D["PROJ"], tag="ones1", name="ones_row")
            nc.sync.dma_start(ones_row[:], ones1[:, :])
            ident = cpool.tile([128, 128], D["TP"], tag="id", name="ident")
            make_identity(nc, ident[:])

            def body():
                for b in range(BPC):
                    # ---- load x^T ----
                    xt = []
                    for ct in range(CT):
                        t_ = xpool.tile([128, NP], D["X"], tag=f"xt{ct}", name=f"xt{ct}")
                        nc.sync.dma_start(t_[:, :N], xT[b, ct * 128 : (ct + 1) * 128, :])
                        xt.append(t_)
                    # ---- q^T, k^T  ([d,n] layout, head pair per 128-tile) ----
                    qt, kt = [], []
                    for name, w, dst in (("q", wq, qt), ("k", wk, kt)) if "qkv" not in ablate else ():
                        for dtt in range(CT):
                            o = qkpool.tile([128, NP], D["QK"], tag=f"{name}{dtt}", name=f"{name}t{dtt}")
                            for c0, cl in XQC:
                                ps = ps_io.tile([128, 512], f32, tag="io", name="iops")
                                for ct in range(CT):
                                    nc.tensor.matmul(
                                        ps[:, :cl],
                                        lhsT=cast(w[ct][:, dtt * 128 : dtt * 128 + 128]),
                                        rhs=cast(xt[ct][:, c0 : c0 + cl]),
                                        start=(ct == 0),
                                        stop=(ct == CT - 1),
                                    )
                                nc.vector.tensor_copy(o[:, c0 : c0 + cl], ps[:, :cl])
                            dst.append(o)
                    # ---- V (normal [n, head*hd] layout + ones column per head) ----
                    va = []
                    for ti, (t0, rows) in enumerate(TOK):
                        if "qkv" in ablate:
                            t_ = vapool.tile([128, H, HD + 2], D["ATT"], tag=f"va{ti}", name=f"va{ti}")
                            va.append(t_)
                            continue
                        t_ = vapool.tile([128, H, HD + 2], D["ATT"], tag=f"va{ti}", name=f"va{ti}")
                        nc.sync.dma_start(t_[:rows, :, HD : HD + 1], onesH[:rows, :])
                        for half, (m0, ml) in enumerate(PC):
                            ps = ps_io.tile([128, 512], f32, tag="io", name="iops")
                            for ct in range(CT):
                                nc.tensor.matmul(
                                    ps[:rows, :ml],
                                    lhsT=cast(xt[ct][:, t0 : t0 + rows]),
                                    rhs=cast(wv[ct][:, m0 : m0 + ml]),
                                    start=(ct == 0),
                                    stop=(ct == CT - 1),
                                )
                            nc.scalar.copy(
                                t_[:rows, 6 * half : 6 * half + 6, :HD],
                                ps[:rows, :ml].rearrange("p (h d) -> p h d", d=HD),
                            )
                        va.append(t_)
                    # ---- head pairs ----
                    zc = [
                        zcpool.tile([128, N], D["PROJ"], tag=f"zc{ct}", name=f"zc{ct}")
                        for ct in range(CT)
                    ]
                    for p in range(PAIRS):
                        if "pairs" in ablate:
                            break
                        # E^T = exp(scale * K Q^T), per head of the pair.
                        # Separate 2-bank psum tiles per (jt, head) with
                        # bufs=2 so head B's logits matmuls overlap head A's
                        # exp on ACT.
                        ets = []
                        for jt, (j0, jrows) in enumerate(TOK):
                            epair = []
                            for h01 in range(2):
                                lo = 64 * h01
                                eps = ps_et.tile([128, 1024], f32, tag="et", name="eps")
                                for i0, il in ETC:
                                    nc.tensor.matmul(
                                        eps[:jrows, i0 : i0 + il],
                                        lhsT=cast(kt[p][lo : lo + 64, j0 : j0 + jrows]),
                                        rhs=cast(qt[p][lo : lo + 64, i0 : i0 + il]),
                                        start=True,
                                        stop=True,
                                    )
                                esb = etpool.tile(
                                    [128, NP], D["ATT"], tag=f"et{jt}_{h01}", name=f"esb{jt}_{h01}"
                                )
                                nc.scalar.activation(
                                    esb[:jrows, :], eps[:jrows, :NP], Exp, scale=SCALE
                                )
                                epair.append(esb)
                            ets.append(epair)
                        ohs = [
                            hspool.tile([128, 128], D["TP"], tag=f"oh{it}", name=f"oh{it}")
                            for it in range(len(TOK))
                        ]
                        for h01 in range(2 * ("uz" not in ablate)):
                            head = 2 * p + h01
                            c1a = cst_sb[:, 0, head : head + 1]
                            c3a = cst_sb[:, 1, head : head + 1]
                            nit = len(TOK)
                            # U_aug = E @ [V | 1] -> U, s. Round-major (jt outer)
                            # so each round's matmuls run as soon as exp(jt) is
                            # done; all 5 token tiles accumulate in ONE psum
                            # bank ([128, 5, 66]).
                            ups = ps_u.tile([128, nit, HD + 2], f32, tag="u", name="ups")
                            ujts = [0, 2, 4] if "skipjt" in probe else range(len(TOK))
                            for jt in ujts:
                                (j0, jrows) = TOK[jt]
                                for it, (t0, rows) in enumerate(TOK):
                                    nc.tensor.matmul(
                                        ups[:rows, it, : (2 if "tinyuz" in probe else HD + 2)],
                                        lhsT=cast(ets[jt][h01][:jrows, t0 : t0 + rows]),
                                        rhs=cast(va[jt][:jrows, head, : (2 if "tinyuz" in probe else HD + 2)]),
                                        start=(jt == ujts[0] and it == 0),
                                        stop=(jt == ujts[-1]),
                                    )
                            ws, srs = [], []
                            for it, (t0, rows) in enumerate(TOK):
                                sr = hspool.tile([128, 1], f32, tag=f"sr{h01}_{it}", name=f"sr{h01}_{it}")
                                nc.vector.reciprocal(sr[:rows], va[it][:rows, head, :1] if "detach" in probe else ups[:rows, it, HD : HD + 1])
                                pp = hspool.tile([128, HD], D["ATT"], tag="pp", name="pp")
                                nc.vector.tensor_scalar(
                                    pp[:rows],
                                    va[it][:rows, head, :HD] if "detach" in probe else ups[:rows, it, :HD],
                                    sr[:rows],
                                    c3a[:rows],
                                    op0=mult,
                                    op1=mult,
                                )
                                w_ = hspool.tile([128, HD], D["ATT"], tag=f"wj{it}", name=f"wj{it}")
                                nc.vector.scalar_tensor_tensor(
                                    w_[:rows],
                                    va[it][:rows, head, :HD],
                                    c1a[:rows],
                                    pp[:rows],
                                    op0=mult,
                                    op1=add,
                                )
                                ws.append(w_)
                                srs.append(sr)
                            if "z" in ablate:
                                continue
                            # Z = E @ W ; out_head = Z / s
                            zps = ps_z.tile([128, nit, HD], f32, tag="z", name="zps")
                            for jt in ujts:
                                (j0, jrows) = TOK[jt]
                                for it, (t0, rows) in enumerate(TOK):
                                    nc.tensor.matmul(
                                        zps[:rows, it, : (2 if "tinyuz" in probe else HD)],
                                        lhsT=cast(ets[jt][h01][:jrows, t0 : t0 + rows]),
                                        rhs=cast(ws[jt][:jrows, : (2 if "tinyuz" in probe else HD)]),
                                        start=(jt == ujts[0] and it == 0),
                                        stop=(jt == ujts[-1]),
                                    )
                            for it, (t0, rows) in enumerate(TOK):
                                nc.vector.tensor_scalar_mul(
                                    ohs[it][:rows, 64 * h01 : 64 * h01 + HD],
                                    va[it][:rows, head, :HD] if "detach" in probe else zps[:rows, it, :],
                                    srs[it][:rows],
                                )
                        # transpose pair -> zcat^T rows
                        for it, (t0, rows) in enumerate(TOK):
                            if "tp" in ablate:
                                break
                            tps = ps_io.tile([128, 128], D["TP"], tag="io", name="tps")
                            nc.tensor.transpose(
                                tps[:, :rows], ohs[it][:rows, :], ident[:rows, :rows]
                            )
                            nc.vector.tensor_copy(zc[p][:, t0 : t0 + rows], tps[:, :rows])
                    # ---- projection + bias, then store ----
                    for it, (t0, rows) in enumerate(TOK):
                        if "proj" in ablate:
                            break
                        ob = obpool.tile([128, C], f32, tag="ob", name="ob")
                        for half, (m0, ml) in enumerate(PC):
                            pps = ps_io.tile([128, 512], f32, tag="io", name="iops")
                            for ct in range(CT):
                                nc.tensor.matmul(
                                    pps[:rows, :ml],
                                    lhsT=cast(zc[ct][:, t0 : t0 + rows]),
                                    rhs=cast(wp[ct][:, m0 : m0 + ml]),
                                    start=(ct == 0),
                                    stop=(ct == CT - 1),
                                )
                            nc.scalar.copy(ob[:rows, m0 : m0 + ml], pps[:rows, :ml])
                        nc.sync.dma_start(out[b, t0 : t0 + rows, :], ob[:rows, :])

            if loop_n > 1:
                with tc.For_i(0, loop_n, 1):
                    body()
            else:
                body()

    nc.compile()
    return nc


def _prep_in_maps(mode, x, Wq, Wk, Wv, Wp, bp, lamb):
    import ml_dtypes

    D = _dtypes(mode)
    bf16 = ml_dtypes.bfloat16

    def npdt(d):
        import concourse.mybir as mybir

        return bf16 if d == mybir.dt.bfloat16 else np.float32

    npW, npX, npPROJ = npdt(D["W"]), npdt(D["X"]), npdt(D["PROJ"])
    wqT = np.ascontiguousarray(Wq.T).astype(npW)
    wkT = np.ascontiguousarray(Wk.T).astype(npW)
    wvT = np.ascontiguousarray(Wv.T).astype(npW)
    wpT = np.ascontiguousarray(Wp.T).astype(npPROJ)
    bpr = np.ascontiguousarray(bp.reshape(1, C)).astype(npPROJ)
    c1 = (1.0 - 2.0 * lamb).astype(np.float32)
    c3 = (3.0 * lamb).astype(np.float32)
    cstv = np.ascontiguousarray(
        np.broadcast_to(np.stack([c1, c3], 0)[None], (128, 2, H))
    ).astype(np.float32)
    onesHv = np.ones((128, H), dtype=npdt(D["ATT"]))
    ones1v = np.ones((1, 128), dtype=npPROJ)
    in_maps = []
    for core in range(NCORES):
        xs = x[core * BPC : (core + 1) * BPC]
        xTv = np.ascontiguousarray(xs.transpose(0, 2, 1)).astype(npX)
        in_maps.append(
            dict(xT=xTv, wqT=wqT, wkT=wkT, wvT=wvT, wpT=wpT, bpr=bpr, cst=cstv,
                 onesH=onesHv, ones1=ones1v)
        )
    return in_maps


def kernel(x, Wq, Wk, Wv, Wp, bp, lamb):
    from concourse.bass_utils import run_bass_kernel_spmd

    x = np.asarray(x, dtype=np.float32)
    Wq = np.asarray(Wq, dtype=np.float32)
    Wk = np.asarray(Wk, dtype=np.float32)
    Wv = np.asarray(Wv, dtype=np.float32)
    Wp = np.asarray(Wp, dtype=np.float32)
    bp = np.asarray(bp, dtype=np.float32)
    lamb = np.asarray(lamb, dtype=np.float32)

    if MODE not in _CACHE:
        _CACHE[MODE] = build(MODE)
    nc = _CACHE[MODE]
    in_maps = _prep_in_maps(MODE, x, Wq, Wk, Wv, Wp, bp, lamb)
    res = run_bass_kernel_spmd(nc, in_maps, list(range(NCORES)))
    out = np.concatenate([res.results[i]["out"] for i in range(NCORES)], axis=0)
    out += bp[None, None, :]
    return out


# revision 24
# speedup vs baseline: 10.5427x; 1.0792x over previous
"""Class-attention with GFSA reaction term — Trainium2 Bass kernel, 8 NeuronCores.

Math (reference):
    q,k,v = x@W{q,k,v}.T split into H=12 heads of 64
    A  = softmax(q k^T / 8)                  per (b,h), [N,N], N=577
    attn = A + lamb_h * (3*A@A - 2*A)
    out  = (attn @ v) @ Wp.T + bp

Key algebraic restructure (avoids the N^3 A@A entirely):
    out_head = (1-2l)*A@V + 3l*A@(A@V)
    With E = exp(logits) unnormalized and D = diag(rowsum(E)):
        U = E@V,  s = rowsum(E)  (one fused matmul with a ones-column on V)
        W = (1-2l)*V + 3l*diag(1/s)*U
        out_head = diag(1/s) * (E @ W)
    No softmax max-subtraction needed: logits ~ N(0,1) by construction.

Sharding: pure data-parallel over batch, 2 batches per core, no collectives.
All layout transposes are done host-side in numpy (x^T, W^T fed directly).
"""

import os
import sys

import numpy as np

for _p in ("/opt/trn_rl_repo", "/root/.axon_site", "/root/.axon_site/_ro/trn_rl_repo"):
    if _p not in sys.path and os.path.isdir(_p):
        sys.path.append(_p)

B, N, C, H, HD = 16, 577, 768, 12, 64
NCORES = 8
BPC = B // NCORES  # batches per core
SCALE = HD**-0.5
PAIRS = H // 2
CT = C // 128  # 6 c-tiles

# token tiles (start, rows)
TOK = [(t * 128, min(128, N - t * 128)) for t in range((N + 127) // 128)]
NP = N + 1  # 578: fp32r matmuls need an even moving free dim, so pad tokens
ETC = [(0, 512), (512, NP - 512)]  # ET matmul free chunks (bank-aligned, even)
XQC = [(0, 512), (512, NP - 512)]  # q/k projection free chunks (even)
PC = [(0, 384), (384, 384)]  # v / proj free chunks

MODE = os.environ.get("KERNEL_MODE", "bf16")

_CACHE = {}


def _dtypes(mode):
    import concourse.mybir as mybir

    f32, f32r, bf16 = mybir.dt.float32, mybir.dt.float32r, mybir.dt.bfloat16
    if mode == "bf16":
        return dict(W=bf16, X=bf16, QK=bf16, ATT=bf16, TP=bf16, PROJ=bf16)
    if mode == "mixed":
        return dict(W=f32r, X=f32r, QK=f32r, ATT=bf16, TP=bf16, PROJ=bf16)
    if mode == "fp32r":
        return dict(W=f32r, X=f32r, QK=f32r, ATT=f32r, TP=f32, PROJ=f32r)
    if mode == "fp32":
        return dict(W=f32, X=f32, QK=f32, ATT=f32, TP=f32, PROJ=f32)
    raise ValueError(mode)


def build(mode=MODE, loop_n=1, ablate=(), probe=()):
    """Build + compile the per-core Bass program (SPMD, identical on all cores)."""
    import concourse.mybir as mybir
    import concourse.tile as tile
    from concourse import bacc
    from concourse.masks import make_identity

    D = _dtypes(mode)
    f32 = mybir.dt.float32
    Exp = mybir.ActivationFunctionType.Exp
    mult, add = mybir.AluOpType.mult, mybir.AluOpType.add

    def cast(ap):
        return ap

    nc = bacc.Bacc("TRN2", target_bir_lowering=False, debug=False, num_devices=NCORES)

    xT = nc.dram_tensor("xT", [BPC, C, N], D["X"], kind="ExternalInput")
    wqT = nc.dram_tensor("wqT", [C, C], D["W"], kind="ExternalInput")
    wkT = nc.dram_tensor("wkT", [C, C], D["W"], kind="ExternalInput")
    wvT = nc.dram_tensor("wvT", [C, C], D["W"], kind="ExternalInput")
    wpT = nc.dram_tensor("wpT", [C, C], D["PROJ"], kind="ExternalInput")
    bpr = nc.dram_tensor("bpr", [1, C], D["PROJ"], kind="ExternalInput")
    cst = nc.dram_tensor("cst", [128, 2, H], f32, kind="ExternalInput")
    onesH = nc.dram_tensor("onesH", [128, H], D["ATT"], kind="ExternalInput")
    ones1 = nc.dram_tensor("ones1", [1, 128], D["PROJ"], kind="ExternalInput")
    out = nc.dram_tensor("out", [BPC, N, C], f32, kind="ExternalOutput")

    with tile.TileContext(nc) as tc:
        # per-mode buffer depths (SBUF budget: fp32 tiles are 2x bigger)
        big = mode == "bf16"
        xb = 2 if big else 1
        qb = 2 if big else 1
        vb = 2 if mode in ("bf16", "mixed") else 1
        eb = 2 if mode in ("bf16", "mixed") else 1
        zb = 2 if mode in ("bf16", "mixed") else 1
        with (
            tc.tile_pool(name="wp", bufs=1) as wpool,
            tc.tile_pool(name="cp", bufs=1) as cpool,
            tc.tile_pool(name="xp", bufs=xb) as xpool,
            tc.tile_pool(name="qkp", bufs=qb) as qkpool,
            tc.tile_pool(name="vap", bufs=vb) as vapool,
            tc.tile_pool(name="etp", bufs=eb + 1) as etpool,
            tc.tile_pool(name="hsp", bufs=3) as hspool,
            tc.tile_pool(name="zcp", bufs=zb) as zcpool,
            tc.tile_pool(name="obp", bufs=3) as obpool,
            tc.tile_pool(name="ps_et", bufs=2, space="PSUM") as ps_et,
            tc.tile_pool(name="ps_u", bufs=1, space="PSUM") as ps_u,
            tc.tile_pool(name="ps_z", bufs=1, space="PSUM") as ps_z,
            tc.tile_pool(name="ps_io", bufs=2, space="PSUM") as ps_io,
        ):
            # ---- persistent constants / weights ----
            wq = [wpool.tile([128, C], D["W"], tag=f"wq{ct}", name=f"wq{ct}") for ct in range(CT)]
            wk = [wpool.tile([128, C], D["W"], tag=f"wk{ct}", name=f"wk{ct}") for ct in range(CT)]
            wv = [wpool.tile([128, C], D["W"], tag=f"wv{ct}", name=f"wv{ct}") for ct in range(CT)]
            wp = [wpool.tile([128, C], D["PROJ"], tag=f"wp{ct}", name=f"wp{ct}") for ct in range(CT)]
            # weights on the scalar HWDGE queue so they stream in parallel
            # with the x^T loads issued on the sync queue inside body()
            for ct in range(CT):
                sl = slice(ct * 128, (ct + 1) * 128)
                nc.scalar.dma_start(wq[ct][:], wqT[sl, :])
                nc.scalar.dma_start(wk[ct][:], wkT[sl, :])
            for ct in range(CT):
                sl = slice(ct * 128, (ct + 1) * 128)
                nc.scalar.dma_start(wv[ct][:], wvT[sl, :])
                nc.scalar.dma_start(wp[ct][:], wpT[sl, :])
            cst_sb = cpool.tile([128, 2, H], f32, tag="cst", name="cst_sb")
            nc.sync.dma_start(cst_sb[:], cst[:, :, :])
            bp_sb = cpool.tile([1, C], D["PROJ"], tag="bp", name="bp_sb")
            nc.sync.dma_start(bp_sb[:], bpr[:, :])
            ones_row = cpool.tile([1, 128], D["PROJ"], tag="ones1", name="ones_row")
            nc.sync.dma_start(ones_row[:], ones1[:, :])
            ident = cpool.tile([128, 128], D["TP"], tag="id", name="ident")
            make_identity(nc, ident[:])

            def body():
                QT, KT, VA, ZC = {}, {}, {}, {}
                for b in range(BPC):
                    # ---- load x^T ----
                    xt = []
                    for ct in range(CT):
                        t_ = xpool.tile([128, NP], D["X"], tag=f"xt{ct}", name=f"xt{ct}")
                        nc.sync.dma_start(t_[:, :N], xT[b, ct * 128 : (ct + 1) * 128, :])
                        xt.append(t_)
                    # ---- q^T, k^T  ([d,n] layout, head pair per 128-tile) ----
                    qt, kt = [], []
                    for name, w, dst in (("q", wq, qt), ("k", wk, kt)) if "qkv" not in ablate else ():
                        for dtt in range(CT):
                            o = qkpool.tile([128, NP], D["QK"], tag=f"{name}{dtt}", name=f"{name}t{dtt}")
                            for c0, cl in XQC:
                                ps = ps_io.tile([128, 512], f32, tag="io", name="iops")
                                for ct in range(CT):
                                    nc.tensor.matmul(
                                        ps[:, :cl],
                                        lhsT=cast(w[ct][:, dtt * 128 : dtt * 128 + 128]),
                                        rhs=cast(xt[ct][:, c0 : c0 + cl]),
                                        start=(ct == 0),
                                        stop=(ct == CT - 1),
                                    )
                                nc.vector.tensor_copy(o[:, c0 : c0 + cl], ps[:, :cl])
                            dst.append(o)
                    # ---- V (normal [n, head*hd] layout + ones column per head) ----
                    va = []
                    for ti, (t0, rows) in enumerate(TOK):
                        if "qkv" in ablate:
                            t_ = vapool.tile([128, H, HD + 2], D["ATT"], tag=f"va{ti}", name=f"va{ti}")
                            va.append(t_)
                            continue
                        t_ = vapool.tile([128, H, HD + 2], D["ATT"], tag=f"va{ti}", name=f"va{ti}")
                        nc.sync.dma_start(t_[:rows, :, HD : HD + 1], onesH[:rows, :])
                        for half, (m0, ml) in enumerate(PC):
                            ps = ps_io.tile([128, 512], f32, tag="io", name="iops")
                            for ct in range(CT):
                                nc.tensor.matmul(
                                    ps[:rows, :ml],
                                    lhsT=cast(xt[ct][:, t0 : t0 + rows]),
                                    rhs=cast(wv[ct][:, m0 : m0 + ml]),
                                    start=(ct == 0),
                                    stop=(ct == CT - 1),
                                )
                            nc.scalar.copy(
                                t_[:rows, 6 * half : 6 * half + 6, :HD],
                                ps[:rows, :ml].rearrange("p (h d) -> p h d", d=HD),
                            )
                        va.append(t_)
                    QT[b], KT[b], VA[b] = qt, kt, va
                for b in range(BPC):
                    qt, kt, va = QT[b], KT[b], VA[b]
                    # ---- head pairs ----
                    zc = [
                        zcpool.tile([128, N], D["PROJ"], tag=f"zc{ct}", name=f"zc{ct}")
                        for ct in range(CT)
                    ]
                    ZC[b] = zc
                    for p in range(PAIRS):
                        if "pairs" in ablate:
                            break
                        # E^T = exp(scale * K Q^T), per head of the pair.
                        # Separate 2-bank psum tiles per (jt, head) with
                        # bufs=2 so head B's logits matmuls overlap head A's
                        # exp on ACT.
                        ets = []
                        for jt, (j0, jrows) in enumerate(TOK):
                            epair = []
                            for h01 in range(2):
                                lo = 64 * h01
                                eps = ps_et.tile([128, 1024], f32, tag="et", name="eps")
                                for i0, il in ETC:
                                    nc.tensor.matmul(
                                        eps[:jrows, i0 : i0 + il],
                                        lhsT=cast(kt[p][lo : lo + 64, j0 : j0 + jrows]),
                                        rhs=cast(qt[p][lo : lo + 64, i0 : i0 + il]),
                                        start=True,
                                        stop=True,
                                    )
                                esb = etpool.tile(
                                    [128, NP], D["ATT"], tag=f"et{jt}_{h01}", name=f"esb{jt}_{h01}"
                                )
                                nc.scalar.activation(
                                    esb[:jrows, :], eps[:jrows, :NP], Exp, scale=SCALE
                                )
                                epair.append(esb)
                            ets.append(epair)
                        ohs = [
                            hspool.tile([128, 128], D["TP"], tag=f"oh{it}", name=f"oh{it}")
                            for it in range(len(TOK))
                        ]
                        for h01 in range(2 * ("uz" not in ablate)):
                            head = 2 * p + h01
                            c1a = cst_sb[:, 0, head : head + 1]
                            c3a = cst_sb[:, 1, head : head + 1]
                            nit = len(TOK)
                            # U_aug = E @ [V | 1] -> U, s. Round-major (jt outer)
                            # so each round's matmuls run as soon as exp(jt) is
                            # done; all 5 token tiles accumulate in ONE psum
                            # bank ([128, 5, 66]).
                            ups = ps_u.tile([128, nit, HD + 2], f32, tag="u", name="ups")
                            ujts = [0, 2, 4] if "skipjt" in probe else range(len(TOK))
                            for jt in ujts:
                                (j0, jrows) = TOK[jt]
                                for it, (t0, rows) in enumerate(TOK):
                                    nc.tensor.matmul(
                                        ups[:rows, it, : (2 if "tinyuz" in probe else HD + 2)],
                                        lhsT=cast(ets[jt][h01][:jrows, t0 : t0 + rows]),
                                        rhs=cast(va[jt][:jrows, head, : (2 if "tinyuz" in probe else HD + 2)]),
                                        start=(jt == ujts[0] and it == 0),
                                        stop=(jt == ujts[-1]),
                                    )
                            ws, srs = [], []
                            for it, (t0, rows) in enumerate(TOK):
                                sr = hspool.tile([128, 1], f32, tag=f"sr{h01}_{it}", name=f"sr{h01}_{it}")
                                nc.vector.reciprocal(sr[:rows], va[it][:rows, head, :1] if "detach" in probe else ups[:rows, it, HD : HD + 1])
                                pp = hspool.tile([128, HD], D["ATT"], tag="pp", name="pp")
                                nc.vector.tensor_scalar(
                                    pp[:rows],
                                    va[it][:rows, head, :HD] if "detach" in probe else ups[:rows, it, :HD],
                                    sr[:rows],
                                    c3a[:rows],
                                    op0=mult,
                                    op1=mult,
                                )
                                w_ = hspool.tile([128, HD], D["ATT"], tag=f"wj{it}", name=f"wj{it}")
                                nc.vector.scalar_tensor_tensor(
                                    w_[:rows],
                                    va[it][:rows, head, :HD],
                                    c1a[:rows],
                                    pp[:rows],
                                    op0=mult,
                                    op1=add,
                                )
                                ws.append(w_)
                                srs.append(sr)
                            if "z" in ablate:
                                continue
                            # Z = E @ W ; out_head = Z / s
                            zps = ps_z.tile([128, nit, HD], f32, tag="z", name="zps")
                            for jt in ujts:
                                (j0, jrows) = TOK[jt]
                                for it, (t0, rows) in enumerate(TOK):
                                    nc.tensor.matmul(
                                        zps[:rows, it, : (2 if "tinyuz" in probe else HD)],
                                        lhsT=cast(ets[jt][h01][:jrows, t0 : t0 + rows]),
                                        rhs=cast(ws[jt][:jrows, : (2 if "tinyuz" in probe else HD)]),
                                        start=(jt == ujts[0] and it == 0),
                                        stop=(jt == ujts[-1]),
                                    )
                            for it, (t0, rows) in enumerate(TOK):
                                nc.vector.tensor_scalar_mul(
                                    ohs[it][:rows, 64 * h01 : 64 * h01 + HD],
                                    va[it][:rows, head, :HD] if "detach" in probe else zps[:rows, it, :],
                                    srs[it][:rows],
                                )
                        # transpose pair -> zcat^T rows
                        for it, (t0, rows) in enumerate(TOK):
                            if "tp" in ablate:
                                break
                            tps = ps_io.tile([128, 128], D["TP"], tag="io", name="tps")
                            nc.tensor.transpose(
                                tps[:, :rows], ohs[it][:rows, :], ident[:rows, :rows]
                            )
                            nc.vector.tensor_copy(zc[p][:, t0 : t0 + rows], tps[:, :rows])
                for b in range(BPC):
                    zc = ZC[b]
                    # ---- projection, then store (bias added host-side) ----
                    for it, (t0, rows) in enumerate(TOK):
                        if "proj" in ablate:
                            break
                        ob = obpool.tile([128, C], f32, tag="ob", name="ob")
                        for half, (m0, ml) in enumerate(PC):
                            pps = ps_io.tile([128, 512], f32, tag="io", name="iops")
                            for ct in range(CT):
                                nc.tensor.matmul(
                                    pps[:rows, :ml],
                                    lhsT=cast(zc[ct][:, t0 : t0 + rows]),
                                    rhs=cast(wp[ct][:, m0 : m0 + ml]),
                                    start=(ct == 0),
                                    stop=(ct == CT - 1),
                                )
                            nc.scalar.copy(ob[:rows, m0 : m0 + ml], pps[:rows, :ml])
                        nc.sync.dma_start(out[b, t0 : t0 + rows, :], ob[:rows, :])

            if loop_n > 1:
                with tc.For_i(0, loop_n, 1):
                    body()
            else:
                body()

    nc.compile()
    return nc


def _prep_in_maps(mode, x, Wq, Wk, Wv, Wp, bp, lamb):
    import ml_dtypes

    D = _dtypes(mode)
    bf16 = ml_dtypes.bfloat16

    def npdt(d):
        import concourse.mybir as mybir

        return bf16 if d == mybir.dt.bfloat16 else np.float32

    npW, npX, npPROJ = npdt(D["W"]), npdt(D["X"]), npdt(D["PROJ"])
    wqT = np.ascontiguousarray(Wq.T).astype(npW)
    wkT = np.ascontiguousarray(Wk.T).astype(npW)
    wvT = np.ascontiguousarray(Wv.T).astype(npW)
    wpT = np.ascontiguousarray(Wp.T).astype(npPROJ)
    bpr = np.ascontiguousarray(bp.reshape(1, C)).astype(npPROJ)
    c1 = (1.0 - 2.0 * lamb).astype(np.float32)
    c3 = (3.0 * lamb).astype(np.float32)
    cstv = np.ascontiguousarray(
        np.broadcast_to(np.stack([c1, c3], 0)[None], (128, 2, H))
    ).astype(np.float32)
    onesHv = np.ones((128, H), dtype=npdt(D["ATT"]))
    ones1v = np.ones((1, 128), dtype=npPROJ)
    in_maps = []
    for core in range(NCORES):
        xs = x[core * BPC : (core + 1) * BPC]
        xTv = np.ascontiguousarray(xs.transpose(0, 2, 1)).astype(npX)
        in_maps.append(
            dict(xT=xTv, wqT=wqT, wkT=wkT, wvT=wvT, wpT=wpT, bpr=bpr, cst=cstv,
                 onesH=onesHv, ones1=ones1v)
        )
    return in_maps


def kernel(x, Wq, Wk, Wv, Wp, bp, lamb):
    from concourse.bass_utils import run_bass_kernel_spmd

    x = np.asarray(x, dtype=np.float32)
    Wq = np.asarray(Wq, dtype=np.float32)
    Wk = np.asarray(Wk, dtype=np.float32)
    Wv = np.asarray(Wv, dtype=np.float32)
    Wp = np.asarray(Wp, dtype=np.float32)
    bp = np.asarray(bp, dtype=np.float32)
    lamb = np.asarray(lamb, dtype=np.float32)

    if MODE not in _CACHE:
        _CACHE[MODE] = build(MODE)
    nc = _CACHE[MODE]
    in_maps = _prep_in_maps(MODE, x, Wq, Wk, Wv, Wp, bp, lamb)
    res = run_bass_kernel_spmd(nc, in_maps, list(range(NCORES)))
    out = np.concatenate([res.results[i]["out"] for i in range(NCORES)], axis=0)
    out += bp[None, None, :]
    return out
